# revision 1
# baseline (speedup 1.0000x reference)
"""Raw-bass v4: minimal instructions + minimal input bytes (bf16 inputs).

Same math as v3; changes:
  - xTroll / thp inputs are bf16 (halves host->device transfer)
  - thpT input dropped: ssq computed as 40 tiny PE matmuls
    ssq_psum[:, d] += th2[:, i, dblock]^T @ ones  (has_written per-column)
  - sqrt reads ssq directly from PSUM
"""

import numpy as np
from contextlib import ExitStack

B, NIN, NK, DK = 512, 1024, 128, 5
NCORES = 8
BL = B // NCORES
P = 128
IT = NIN // P
R = 16
NBLK = BL // R


def build_nc(repeat=1):
    import concourse.bacc as bacc
    import concourse.mybir as mybir

    f32 = mybir.dt.float32
    bf16 = mybir.dt.bfloat16
    AF = mybir.ActivationFunctionType
    OP = mybir.AluOpType
    X = mybir.AxisListType.X

    nc = bacc.Bacc(None, target_bir_lowering=False)
    xT_d = nc.declare_dram_parameter("xTroll", [NIN, B], bf16, isOutput=False)
    thp_d = nc.declare_dram_parameter("thp", [NIN, DK * NK + 1], bf16, isOutput=False)
    small_d = nc.declare_dram_parameter("small", [NK, DK + 1], f32, isOutput=False)
    fT_d = nc.declare_dram_parameter("fT", [NK, BL], f32, isOutput=True)

    with ExitStack() as ctx:
        en = ctx.enter_context
        th_all = en(nc.sbuf_tensor([P, IT, DK * NK + 1], bf16))
        xT_all = en(nc.sbuf_tensor([P, IT, B], bf16))
        th2 = en(nc.sbuf_tensor([P, IT, DK * NK + 1], bf16))
        small = en(nc.sbuf_tensor([NK, DK + 1], f32))
        l2c = en(nc.sbuf_tensor([NK, DK], f32))
        invc = en(nc.sbuf_tensor([NK, DK], f32))
        elws = en(nc.sbuf_tensor([NK, DK], f32))
        scale = en(nc.sbuf_tensor([NK, DK], f32))
        bm1 = en(nc.sbuf_tensor([NK, 1], f32))
        avT = en(nc.sbuf_tensor([P, DK, B], bf16))
        fTs = en(nc.sbuf_tensor([NK, BL], f32))
        fT2 = en(nc.sbuf_tensor([NK, BL], f32))
        diff = en(nc.sbuf_tensor([P, DK * R * B], bf16))
        L = en(nc.sbuf_tensor([P, R * B], f32))
        E = en(nc.sbuf_tensor([P, R * B], bf16))
        psums = [en(nc.psum_tensor(f"ps{d}", [P, B], f32)) for d in range(DK)]
        ps_sq = en(nc.psum_tensor("ps_sq", [P, DK], f32))

        d4 = diff[:].rearrange("p (d r b) -> p d r b", r=R, d=DK)
        in0 = avT[:][:, :, None, :].broadcast_to([P, DK, R, B])
        rsrc = diff[:].rearrange("p (d r b) -> p r b d", r=R, d=DK)
        esrc = E[:].rearrange("p (r b) -> p r b", r=R)

        with (
            nc.semaphore("dS") as dS,
            nc.semaphore("sP") as sP,
            nc.semaphore("sA") as sA,
            nc.semaphore("sV") as sV,
            nc.semaphore("s1") as s1,
            nc.semaphore("s2") as s2,
            nc.semaphore("dF") as dF,
            nc.Block() as block,
        ):

            @block.sync
            def _(sync):
                sync.dma_start(
                    th_all[:], thp_d.rearrange("(i p) c -> p i c", p=P)
                ).then_inc(dS, 16)
                sync.dma_start(
                    xT_all[:], xT_d.rearrange("(i p) c -> p i c", p=P)
                ).then_inc(dS, 16)
                sync.dma_start(small[:], small_d[:, :]).then_inc(dS, 16)
                sync.wait_ge(dF, 1)
                sync.dma_start(fT_d[:, :], fT2[:]).then_inc(dS, 16)
                sync.wait_ge(dS, 64)

            @block.tensor
            def _(tensor):
                tensor.wait_ge(dS, 48)  # all loads
                for d in range(DK):
                    for i in range(IT):
                        mm = nc.tensor.matmul(
                            psums[d][:],
                            th_all[:, i, NK * d : NK * (d + 1)],
                            xT_all[:, i, :],
                            start=(i == 0),
                            stop=(i == IT - 1),
                        )
                mm.then_inc(sP, 1)
                tensor.wait_ge(sA, 1)  # th2 ready
                for d in range(DK):
                    for i in range(IT):
                        mm = nc.tensor.matmul(
                            ps_sq[:, d : d + 1],
                            th2[:, i, NK * d : NK * (d + 1)],
                            th_all[:, i, DK * NK : DK * NK + 1],
                            start=(d == 0 and i == 0),
                            stop=(d == DK - 1 and i == IT - 1),
                        )
                mm.then_inc(sP, 1)

            @block.scalar
            def _(scalar):
                scalar.wait_ge(dS, 48)  # all loads
                nc.scalar.activation(th2[:], th_all[:], AF.Square).then_inc(
                    sA, 1
                )
                scalar.wait_ge(sP, 2)  # ssq in psum
                nc.scalar.activation(l2c[:], ps_sq[:], AF.Sqrt).then_inc(sA, 1)
                nc.scalar.activation(
                    elws[:], small[:, 0:DK], AF.Exp
                ).then_inc(sA, 1)
                scalar.wait_ge(sV, 2)  # scale ready
                for d in range(DK):
                    act = nc.scalar.activation(
                        avT[:, d, :],
                        psums[d][:],
                        AF.Copy,
                        scale=scale[:, d : d + 1],
                    )
                act.then_inc(sA, 1)
                for it in range(repeat):
                    for blk in range(NBLK):
                        scalar.wait_ge(s1, it * NBLK + blk + 1)
                        nc.scalar.activation(
                            E[:], L[:], AF.Exp, scale=-1.0
                        ).then_inc(s2, 1)

            @block.vector
            def _(vector):
                vector.wait_ge(dS, 48)
                nc.vector.tensor_scalar_add(
                    bm1[:], small[:, DK : DK + 1], -1.0
                )
                vector.wait_ge(sA, 2)
                nc.vector.reciprocal(invc[:], l2c[:]).then_inc(sV, 1)
                vector.wait_ge(sA, 3)
                nc.vector.tensor_mul(scale[:], elws[:], invc[:]).then_inc(
                    sV, 1
                )
                vector.wait_ge(sA, 4)  # avT ready
                for it in range(repeat):
                    for blk in range(NBLK):
                        r0 = blk * R
                        in1 = avT[:][:, :, r0 : r0 + R, None].broadcast_to(
                            [P, DK, R, B]
                        )
                        nc.vector.tensor_tensor(
                            out=d4, in0=in0, in1=in1, op=OP.subtract
                        )
                        nc.vector.tensor_reduce(
                            L[:],
                            rsrc,
                            axis=X,
                            op=OP.add,
                            apply_absolute_value=True,
                        ).then_inc(s1, 1)
                        vector.wait_ge(s2, it * NBLK + blk + 1)
                        nc.vector.tensor_reduce(
                            fTs[:, r0 : r0 + R],
                            esrc,
                            axis=X,
                            op=OP.add,
                        )
                nc.vector.tensor_scalar_add(
                    fT2[:], fTs[:], bm1[:, 0:1]
                ).then_inc(dF, 1)

    nc.compile()
    return nc


def make_in_maps(x, theta, log_weight_scale, bias):
    import ml_dtypes

    bf = ml_dtypes.bfloat16
    xT = np.ascontiguousarray(x.T).astype(bf)
    thp = (
        np.ascontiguousarray(theta.transpose(0, 2, 1))
        .reshape(NIN, DK * NK)
        .astype(bf)
    )
    thp = np.concatenate([thp, np.ones((NIN, 1), dtype=bf)], axis=1)
    small = np.concatenate(
        [
            np.ascontiguousarray(log_weight_scale).astype(np.float32),
            np.ascontiguousarray(bias.reshape(NK, 1)).astype(np.float32),
        ],
        axis=1,
    )
    return [
        {
            "xTroll": np.ascontiguousarray(np.roll(xT, -BL * c, axis=1)),
            "thp": thp,
            "small": small,
        }
        for c in range(NCORES)
    ]


_CACHE = {}


def get_nc():
    if "nc" not in _CACHE:
        _CACHE["nc"] = build_nc()
    return _CACHE["nc"]


def kernel(x, theta, log_weight_scale, bias):
    from concourse.bass_utils import run_bass_kernel_spmd

    x = np.asarray(x, dtype=np.float32)
    theta = np.asarray(theta, dtype=np.float32)
    log_weight_scale = np.asarray(log_weight_scale, dtype=np.float32)
    bias = np.asarray(bias, dtype=np.float32)

    nc = get_nc()
    in_maps = make_in_maps(x, theta, log_weight_scale, bias)
    res = run_bass_kernel_spmd(nc, in_maps, list(range(NCORES))).results
    f = np.concatenate(
        [res[c]["fT"].T for c in range(NCORES)], axis=0
    )  # [B, NK]
    return np.concatenate([x, f.astype(np.float32)], axis=1)



# revision 7
# speedup vs baseline: 2.5827x; 2.5827x over previous
"""Raw-bass v5: TS-subtract at 4x + sign-clear abs + ACT abs/exp offload.

Per-core (core c owns rows c*64..c*64+63; columns rolled so own rows sit
at cols 0..63):
  - host pre-normalizes kernel weights (theta * exp(lws) / ||theta||),
    so the device skips the ssq/sqrt/recip chain entirely
  - PE: actv matmuls -> ps[d] (PSUM f32); ACT copies -> avT bf16 (full),
    avF f32 (own-row scalars), avFn4 = -avF[4]
  - main loop, groups of RG=8 rows:
      DVE: per (r, d<4): tensor_scalar(td, avT[:,d,:], avF[:,d,r],
           op0=subtract)                                   (4x mode)
      DVE: one batched bitwise_and 0x7FFF on td as uint16  (abs, 4x)
      ACT: per r: Abs(avT[:,4,:] + (-s4)) -> t4            (fused abs)
      DVE: adds: L = (|y0|+|y1|) + (|y2|+|y3|) + t4        (2x mode)
      ACT: per r: Exp(-L) with accum_out -> fTs[:, r]      (exp + rowsum)
  - tail: fT2 = fTs + (bias - 1)   (self-pair contributes exp(0)=1)
"""

import numpy as np
from contextlib import ExitStack

B, NIN, NK, DK = 512, 1024, 128, 5
NCORES = 8
BL = B // NCORES
P = 128
IT = NIN // P
RG = 8
NG = BL // RG


def build_nc():
    import concourse.bacc as bacc
    import concourse.mybir as mybir

    f32 = mybir.dt.float32
    bf16 = mybir.dt.bfloat16
    u16 = mybir.dt.uint16
    AF = mybir.ActivationFunctionType
    OP = mybir.AluOpType

    nc = bacc.Bacc(None, target_bir_lowering=False)
    xT_d = nc.declare_dram_parameter("xTroll", [NIN, B], bf16, isOutput=False)
    kT_d = nc.declare_dram_parameter("kT", [NIN, DK * NK], bf16, isOutput=False)
    bm1_d = nc.declare_dram_parameter("bm1", [NK, 1], f32, isOutput=False)
    fT_d = nc.declare_dram_parameter("fT", [NK, BL], f32, isOutput=True)

    with ExitStack() as ctx:
        en = ctx.enter_context
        kT_all = en(nc.sbuf_tensor([P, IT, DK * NK], bf16))
        xT_all = en(nc.sbuf_tensor([P, IT, B], bf16))
        bm1 = en(nc.sbuf_tensor([NK, 1], f32))
        avT = en(nc.sbuf_tensor([P, DK, B], bf16))
        avF = en(nc.sbuf_tensor([P, DK, BL], f32))
        avFn4 = en(nc.sbuf_tensor([P, BL], f32))
        td = en(nc.sbuf_tensor([P, RG, 4, B], bf16))
        t4 = en(nc.sbuf_tensor([P, 2, RG, B], bf16))
        ta = en(nc.sbuf_tensor([P, RG, B], bf16))
        tb = en(nc.sbuf_tensor([P, RG, B], bf16))
        tc = en(nc.sbuf_tensor([P, RG, B], bf16))
        L2 = en(nc.sbuf_tensor([P, 2, RG, B], bf16))
        Escr = en(nc.sbuf_tensor([P, B], bf16))
        fTs = en(nc.sbuf_tensor([NK, BL], f32))
        fT2 = en(nc.sbuf_tensor([NK, BL], f32))
        ps = [en(nc.psum_tensor(f"ps{d}", [P, B], f32)) for d in range(DK)]

        with (
            nc.semaphore("dS") as dS,
            nc.semaphore("sP") as sP,
            nc.semaphore("sA") as sA,
            nc.semaphore("sB") as sB,
            nc.semaphore("s1") as s1,
            nc.semaphore("s2") as s2,
            nc.semaphore("dF") as dF,
            nc.Block() as block,
        ):

            @block.sync
            def _(sync):
                sync.dma_start(
                    kT_all[:], kT_d.rearrange("(i p) c -> p i c", p=P)
                ).then_inc(dS, 16)
                sync.dma_start(
                    xT_all[:], xT_d.rearrange("(i p) c -> p i c", p=P)
                ).then_inc(dS, 16)
                sync.dma_start(bm1[:], bm1_d[:, :]).then_inc(dS, 16)
                sync.wait_ge(dF, 1)
                sync.dma_start(fT_d[:, :], fT2[:]).then_inc(dS, 16)
                sync.wait_ge(dS, 64)

            @block.tensor
            def _(tensor):
                tensor.wait_ge(dS, 48)  # all loads
                for d in range(DK):
                    for i in range(IT):
                        mm = nc.tensor.matmul(
                            ps[d][:],
                            kT_all[:, i, NK * d : NK * (d + 1)],
                            xT_all[:, i, :],
                            start=(i == 0),
                            stop=(i == IT - 1),
                        )
                    mm.then_inc(sP, 1)

            @block.scalar
            def _(scalar):
                for d in range(DK):
                    scalar.wait_ge(sP, d + 1)
                    nc.scalar.activation(avT[:, d, :], ps[d][:], AF.Copy)
                    act = nc.scalar.activation(
                        avF[:, d, :], ps[d][:, 0:BL], AF.Copy
                    )
                act = nc.scalar.activation(
                    avFn4[:], ps[DK - 1][:, 0:BL], AF.Copy, scale=-1.0
                )
                act.then_inc(sA, 1)
                for g in range(NG):
                    if g >= 2:
                        scalar.wait_ge(s1, g - 1)  # t4 buffer free
                    for r in range(RG):
                        rr = g * RG + r
                        act = nc.scalar.activation(
                            t4[:, g % 2, r, :],
                            avT[:, DK - 1, :],
                            AF.Abs,
                            bias=avFn4[:, rr : rr + 1],
                        )
                    act.then_inc(sB, 1)
                    if g >= 1:
                        scalar.wait_ge(s1, g)  # L2[(g-1)%2] ready
                        for r in range(RG):
                            rr = (g - 1) * RG + r
                            act = nc.scalar.activation(
                                Escr[:],
                                L2[:, (g - 1) % 2, r, :],
                                AF.Exp,
                                scale=-1.0,
                                accum_out=fTs[:, rr : rr + 1],
                            )
                        act.then_inc(s2, 1)
                scalar.wait_ge(s1, NG)
                for r in range(RG):
                    rr = (NG - 1) * RG + r
                    act = nc.scalar.activation(
                        Escr[:],
                        L2[:, (NG - 1) % 2, r, :],
                        AF.Exp,
                        scale=-1.0,
                        accum_out=fTs[:, rr : rr + 1],
                    )
                act.then_inc(s2, 1)

            @block.vector
            def _(vector):
                vector.wait_ge(sA, 1)  # avT/avF ready
                for g in range(NG):
                    if g >= 2:
                        vector.wait_ge(s2, g - 1)  # L2[g%2] free
                    for r in range(RG):
                        rr = g * RG + r
                        for d in range(4):
                            nc.vector.tensor_scalar(
                                td[:, r, d, :],
                                avT[:, d, :],
                                avF[:, d, rr : rr + 1],
                                None,
                                OP.subtract,
                            )
                    nc.vector.tensor_scalar(
                        td[:].bitcast(u16),
                        td[:].bitcast(u16),
                        0x7FFF,
                        None,
                        OP.bitwise_and,
                    )
                    nc.vector.tensor_tensor(
                        out=ta[:], in0=td[:, :, 0, :], in1=td[:, :, 1, :],
                        op=OP.add,
                    )
                    nc.vector.tensor_tensor(
                        out=tb[:], in0=td[:, :, 2, :], in1=td[:, :, 3, :],
                        op=OP.add,
                    )
                    nc.vector.tensor_tensor(
                        out=tc[:], in0=ta[:], in1=tb[:], op=OP.add
                    )
                    vector.wait_ge(sB, g + 1)  # t4[g%2] ready
                    nc.vector.tensor_tensor(
                        out=L2[:, g % 2, :, :], in0=tc[:],
                        in1=t4[:, g % 2, :, :], op=OP.add,
                    ).then_inc(s1, 1)
                vector.wait_ge(s2, NG)  # all fTs written
                nc.vector.tensor_scalar_add(
                    fT2[:], fTs[:], bm1[:, 0:1]
                ).then_inc(dF, 1)

    nc.compile()
    return nc


def make_in_maps(x, theta, log_weight_scale, bias):
    import ml_dtypes

    bf = ml_dtypes.bfloat16
    xT = np.ascontiguousarray(x.T).astype(bf)
    l2 = np.sqrt(np.sum(theta.astype(np.float64) ** 2, axis=0))  # [K, D]
    kern = theta * (np.exp(log_weight_scale) / l2)[None]  # [NIN, K, D] f32
    kT = (
        np.ascontiguousarray(kern.transpose(0, 2, 1))
        .reshape(NIN, DK * NK)
        .astype(bf)
    )
    bm1 = (bias.reshape(NK, 1) - 1.0).astype(np.float32)
    return [
        {
            "xTroll": np.ascontiguousarray(np.roll(xT, -BL * c, axis=1)),
            "kT": kT,
            "bm1": bm1,
        }
        for c in range(NCORES)
    ]


_CACHE = {}


def get_nc():
    if "nc" not in _CACHE:
        _CACHE["nc"] = build_nc()
    return _CACHE["nc"]


def kernel(x, theta, log_weight_scale, bias):
    from concourse.bass_utils import run_bass_kernel_spmd

    x = np.asarray(x, dtype=np.float32)
    theta = np.asarray(theta, dtype=np.float32)
    log_weight_scale = np.asarray(log_weight_scale, dtype=np.float32)
    bias = np.asarray(bias, dtype=np.float32)

    nc = get_nc()
    in_maps = make_in_maps(x, theta, log_weight_scale, bias)
    res = run_bass_kernel_spmd(nc, in_maps, list(range(NCORES))).results
    f = np.concatenate(
        [res[c]["fT"].T for c in range(NCORES)], axis=0
    )  # [B, NK]
    return np.concatenate([x, f.astype(np.float32)], axis=1)


# revision 12
# speedup vs baseline: 3.3357x; 1.2916x over previous
"""Raw-bass v6: symmetric halving — each pair |b,b'| computed once.

Row b covers pairs (b, b+j mod 512) for j=1..256 ("strip"). Every
unordered pair is covered exactly once, except distance-256 pairs which
both owners compute (host subtracts the double-counted Ecol term).

Per-core (core c owns rows c*64..c*64+63; columns rolled so own rows sit
at cols 0..63; strips span rolled cols 1..319 only -> compute only
NB=322 activation columns):
  - host pre-normalizes kernel weights; device does actv matmuls only
  - ACT copies psum -> avT bf16 [P,DK,322] and avS (shifted by 1 col,
    for 4-byte alignment of even-row strips)
  - main loop, groups of RG=8 rows: per row one TT-subtract of all 5
    d-planes against dup'd per-row scalars (2x mode), one batched
    uint16 sign-clear AND (4x), 4 TT adds -> L; ACT: Exp(-L) writes the
    E-strip into Epad and row-sums into f_own via accum_out
  - diagonal-shift add tree over Epad rows -> f2 (transpose-side sums)
  - outputs: f_own [NK,64] f32, f2 [NK,384] bf16, Ecol [NK,64] bf16
Host: F[:, own rows] += f_own; scatter-add f2 at global col
(c*64 + bufcol - 63) mod 512; subtract Ecol (distance-256 double
count); add bias; concat with x.
"""

import numpy as np
from contextlib import ExitStack

B, NIN, NK, DK = 512, 1024, 128, 5
NCORES = 8
BL = B // NCORES
P = 128
IT = NIN // P
RG = 8
NG = BL // RG
NB = 322          # activation columns needed (strip max col 319, +pad)
SW = 256          # strip width
EW = 384          # Epad row width (data at 64..320)


def build_nc():
    import concourse.bacc as bacc
    import concourse.mybir as mybir

    f32 = mybir.dt.float32
    bf16 = mybir.dt.bfloat16
    u16 = mybir.dt.uint16
    AF = mybir.ActivationFunctionType
    OP = mybir.AluOpType

    nc = bacc.Bacc(None, target_bir_lowering=False)
    xT_d = nc.declare_dram_parameter("xTlin", [P, IT * NB], bf16, isOutput=False)
    kT_d = nc.declare_dram_parameter("kTlin", [P, IT * DK * NK], bf16, isOutput=False)
    fo_d = nc.declare_dram_parameter("fo", [NK, BL], f32, isOutput=True)
    f2_d = nc.declare_dram_parameter("f2", [NK, EW], bf16, isOutput=True)
    ec_d = nc.declare_dram_parameter("ec", [NK, BL], bf16, isOutput=True)

    with ExitStack() as ctx:
        en = ctx.enter_context
        kT_all = en(nc.sbuf_tensor([P, IT, DK * NK], bf16))
        xT_all = en(nc.sbuf_tensor([P, IT, NB], bf16))
        avT = en(nc.sbuf_tensor([P, DK, NB], bf16))
        avS = en(nc.sbuf_tensor([P, DK, 320], bf16))
        dup = en(nc.sbuf_tensor([P, RG, DK, 2], bf16))
        td = en(nc.sbuf_tensor([P, RG, DK, SW], bf16))
        ta = en(nc.sbuf_tensor([P, RG, SW], bf16))
        tb = en(nc.sbuf_tensor([P, RG, SW], bf16))
        tc = en(nc.sbuf_tensor([P, RG, SW], bf16))
        L2 = en(nc.sbuf_tensor([P, 2, RG, SW], bf16))
        Epad = en(nc.sbuf_tensor([P, BL, EW], bf16))
        T1 = en(nc.sbuf_tensor([P, 32, EW], bf16))
        T2 = en(nc.sbuf_tensor([P, 16, EW], bf16))
        f2b = en(nc.sbuf_tensor([P, 1, EW], bf16))
        ecb = en(nc.sbuf_tensor([P, BL], bf16))
        fo = en(nc.sbuf_tensor([NK, BL], f32))
        ps = [en(nc.psum_tensor(f"ps{d}", [P, NB], f32)) for d in range(DK)]

        with (
            nc.semaphore("dS") as dS,
            nc.semaphore("sP") as sP,
            nc.semaphore("sA") as sA,
            nc.semaphore("s1") as s1,
            nc.semaphore("s2") as s2,
            nc.semaphore("dF") as dF,
            nc.Block() as block,
        ):

            @block.sync
            def _(sync):
                sync.dma_start(
                    kT_all[:], kT_d.rearrange("p (i c) -> p i c", i=IT)
                ).then_inc(dS, 16)
                sync.dma_start(
                    xT_all[:], xT_d.rearrange("p (i c) -> p i c", i=IT)
                ).then_inc(dS, 16)
                sync.wait_ge(dF, 1)
                sync.dma_start(fo_d[:, :], fo[:]).then_inc(dS, 16)
                sync.dma_start(
                    f2_d[:, :], f2b[:].rearrange("p a c -> p (a c)")
                ).then_inc(dS, 16)
                sync.dma_start(ec_d[:, :], ecb[:]).then_inc(dS, 16)
                sync.wait_ge(dS, 80)

            @block.tensor
            def _(tensor):
                tensor.wait_ge(dS, 32)  # both loads
                for d in range(DK):
                    for i in range(IT):
                        mm = nc.tensor.matmul(
                            ps[d][:],
                            kT_all[:, i, NK * d : NK * (d + 1)],
                            xT_all[:, i, :],
                            start=(i == 0),
                            stop=(i == IT - 1),
                        )
                    mm.then_inc(sP, 1)

            @block.scalar
            def _(scalar):
                for d in range(DK):
                    scalar.wait_ge(sP, d + 1)
                    nc.scalar.activation(avT[:, d, :], ps[d][:], AF.Copy)
                    act = nc.scalar.activation(
                        avS[:, d, :], ps[d][:, 1:321], AF.Copy
                    )
                act.then_inc(sA, 1)
                for g in range(NG):
                    scalar.wait_ge(s1, g + 1)  # L2[g%2] ready
                    for r in range(RG):
                        rr = g * RG + r
                        act = nc.scalar.activation(
                            Epad[:, rr, 64 : 64 + SW],
                            L2[:, g % 2, r, :],
                            AF.Exp,
                            scale=-1.0,
                            accum_out=fo[:, rr : rr + 1],
                        )
                    act.then_inc(s2, 1)

            @block.vector
            def _(vector):
                # zero Epad margins, tree buffers, f2 (hidden in preamble)
                nc.vector.memset(Epad[:, :, 0:64], 0)
                nc.vector.memset(Epad[:, :, 320:EW], 0)
                nc.vector.memset(T1[:], 0)
                nc.vector.memset(T2[:], 0)
                nc.vector.memset(f2b[:], 0)
                vector.wait_ge(sA, 1)  # avT/avS ready
                for g in range(NG):
                    if g >= 2:
                        vector.wait_ge(s2, g - 1)  # L2[g%2] free
                    g0 = g * RG
                    nc.vector.tensor_copy(
                        dup[:],
                        avT[:, :, g0 : g0 + RG]
                        .rearrange("p d r -> p r d")[:, :, :, None]
                        .broadcast_to([P, RG, DK, 2]),
                    )
                    for r in range(RG):
                        rr = g0 + r
                        if rr % 2 == 0:
                            src = avS[:, :, rr : rr + SW]
                        else:
                            src = avT[:, :, rr + 1 : rr + 1 + SW]
                        nc.vector.tensor_tensor(
                            out=td[:, r, :, :].rearrange(
                                "p d (b j) -> p d b j", j=2
                            ),
                            in0=src.rearrange("p d (b j) -> p d b j", j=2),
                            in1=dup[:, r, :, None, :].broadcast_to(
                                [P, DK, SW // 2, 2]
                            ),
                            op=OP.subtract,
                        )
                    nc.vector.tensor_scalar(
                        td[:].bitcast(u16),
                        td[:].bitcast(u16),
                        0x7FFF,
                        None,
                        OP.bitwise_and,
                    )
                    nc.vector.tensor_tensor(
                        out=ta[:], in0=td[:, :, 0, :], in1=td[:, :, 1, :],
                        op=OP.add,
                    )
                    nc.vector.tensor_tensor(
                        out=tb[:], in0=td[:, :, 2, :], in1=td[:, :, 3, :],
                        op=OP.add,
                    )
                    nc.vector.tensor_tensor(
                        out=tc[:], in0=ta[:], in1=tb[:], op=OP.add
                    )
                    nc.vector.tensor_tensor(
                        out=L2[:, g % 2, :, :], in0=tc[:],
                        in1=td[:, :, 4, :], op=OP.add,
                    ).then_inc(s1, 1)
                vector.wait_ge(s2, NG)  # Epad complete
                # diagonal-shift add tree: f2[c] = sum_r E[r][c - r]
                nc.vector.tensor_tensor(
                    out=T1[:, 0:32, 32:EW], in0=Epad[:, 0:32, 32:EW],
                    in1=Epad[:, 32:64, 0 : EW - 32], op=OP.add,
                )
                nc.vector.tensor_tensor(
                    out=T2[:, 0:16, 16:EW], in0=T1[:, 0:16, 16:EW],
                    in1=T1[:, 16:32, 0 : EW - 16], op=OP.add,
                )
                nc.vector.tensor_tensor(
                    out=T1[:, 0:8, 8:EW], in0=T2[:, 0:8, 8:EW],
                    in1=T2[:, 8:16, 0 : EW - 8], op=OP.add,
                )
                nc.vector.tensor_tensor(
                    out=T2[:, 0:4, 4:EW], in0=T1[:, 0:4, 4:EW],
                    in1=T1[:, 4:8, 0 : EW - 4], op=OP.add,
                )
                nc.vector.tensor_tensor(
                    out=T1[:, 0:2, 2:EW], in0=T2[:, 0:2, 2:EW],
                    in1=T2[:, 2:4, 0 : EW - 2], op=OP.add,
                )
                nc.vector.tensor_tensor(
                    out=f2b[:, :, 1:EW], in0=T1[:, 0:1, 1:EW],
                    in1=T1[:, 1:2, 0 : EW - 1], op=OP.add,
                )
                nc.vector.tensor_copy(
                    ecb[:].rearrange("p (c a) -> p c a", a=1),
                    Epad[:, :, 319:320],
                ).then_inc(dF, 1)

    nc.compile()
    return nc


def make_in_maps(x, theta, log_weight_scale, bias):
    import ml_dtypes

    bf = ml_dtypes.bfloat16
    xT = np.ascontiguousarray(x.T).astype(bf)  # [NIN, B]
    l2 = np.sqrt(np.sum(theta.astype(np.float64) ** 2, axis=0))  # [K, D]
    kern = theta * (np.exp(log_weight_scale) / l2)[None]  # [NIN, K, D] f32
    kT = (
        np.ascontiguousarray(kern.transpose(0, 2, 1))
        .reshape(NIN, DK * NK)
        .astype(bf)
    )
    kTlin = np.ascontiguousarray(
        kT.reshape(IT, P, DK * NK).transpose(1, 0, 2).reshape(P, IT * DK * NK)
    )
    maps = []
    for c in range(NCORES):
        xr = np.roll(xT, -BL * c, axis=1)[:, 0:NB]  # [NIN, NB]
        xlin = np.ascontiguousarray(
            xr.reshape(IT, P, NB).transpose(1, 0, 2).reshape(P, IT * NB)
        )
        maps.append({"xTlin": xlin, "kTlin": kTlin})
    return maps


_CACHE = {}


def get_nc():
    if "nc" not in _CACHE:
        _CACHE["nc"] = build_nc()
    return _CACHE["nc"]


def kernel(x, theta, log_weight_scale, bias):
    from concourse.bass_utils import run_bass_kernel_spmd

    x = np.asarray(x, dtype=np.float32)
    theta = np.asarray(theta, dtype=np.float32)
    log_weight_scale = np.asarray(log_weight_scale, dtype=np.float32)
    bias = np.asarray(bias, dtype=np.float32)

    nc = get_nc()
    in_maps = make_in_maps(x, theta, log_weight_scale, bias)
    res = run_bass_kernel_spmd(nc, in_maps, list(range(NCORES))).results

    F = np.zeros((NK, B), dtype=np.float64)
    cols = np.arange(EW)
    for c in range(NCORES):
        F[:, c * BL : (c + 1) * BL] += res[c]["fo"].astype(np.float64)
        g = (c * BL + (cols - 63)) % B
        np.add.at(F, (slice(None), g), res[c]["f2"].astype(np.float64))
        F[:, c * BL : (c + 1) * BL] -= res[c]["ec"].astype(np.float64)
    f = F.T + bias[None, :]  # [B, NK]
    return np.concatenate([x, f.astype(np.float32)], axis=1)


# revision 14
# speedup vs baseline: 3.9367x; 1.1802x over previous
"""Raw-bass v6.1: symmetric halving + engine rebalance.

Same math as v6 (strips j=1..256 per row; every unordered pair computed
once; distance-256 double count fixed on host via Ecol). Changes:
  - all zero-init moved to ACT (memzero) — DVE loop starts at sA
  - subtracts batched 2 ops/group (even/odd rows) with hand-built
    overlapping-window APs (alignment: even rows read the avS shifted
    copy) — d-planes 0..3 on DVE
  - plane 4 |x - s| on ACT (Abs activation with negated [P,1] bias)
  - diagonal-shift tree level 1 pipelined into loop groups 5..7
Host: scatter-add combine as in v6.
"""

import dataclasses
import numpy as np
from contextlib import ExitStack

B, NIN, NK, DK = 512, 1024, 128, 5
NCORES = 8
BL = B // NCORES
P = 128
IT = NIN // P
RG = 8
NG = BL // RG
NB = 322          # activation columns needed (strip max col 319, +pad)
SW = 256          # strip width
EW = 384          # Epad row width (data at 64..320)
ND = DK - 1       # d-planes handled on DVE


def _win(base, offset, dims):
    return dataclasses.replace(base, ap=[base.ap[0]] + dims, offset=offset)


def build_nc():
    import concourse.bacc as bacc
    import concourse.mybir as mybir

    f32 = mybir.dt.float32
    bf16 = mybir.dt.bfloat16
    u16 = mybir.dt.uint16
    AF = mybir.ActivationFunctionType
    OP = mybir.AluOpType

    nc = bacc.Bacc(None, target_bir_lowering=False)
    xT_d = nc.declare_dram_parameter("xTlin", [P, IT * NB], bf16, isOutput=False)
    kT_d = nc.declare_dram_parameter("kTlin", [P, IT * DK * NK], bf16, isOutput=False)
    fo_d = nc.declare_dram_parameter("fo", [NK, BL], f32, isOutput=True)
    f2_d = nc.declare_dram_parameter("f2", [NK, EW], bf16, isOutput=True)
    ec_d = nc.declare_dram_parameter("ec", [NK, BL], bf16, isOutput=True)

    with ExitStack() as ctx:
        en = ctx.enter_context
        kT_all = en(nc.sbuf_tensor([P, IT, DK * NK], bf16))
        xT_all = en(nc.sbuf_tensor([P, IT, NB], bf16))
        avT = en(nc.sbuf_tensor([P, DK, NB], bf16))
        avS = en(nc.sbuf_tensor([P, DK, 320], bf16))
        avFn4 = en(nc.sbuf_tensor([P, BL], f32))
        dup = en(nc.sbuf_tensor([P, RG, ND, 2], bf16))
        td = en(nc.sbuf_tensor([P, RG, ND, SW], bf16))
        t4 = en(nc.sbuf_tensor([P, 2, RG, SW], bf16))
        ta = en(nc.sbuf_tensor([P, RG, SW], bf16))
        tb = en(nc.sbuf_tensor([P, RG, SW], bf16))
        tc = en(nc.sbuf_tensor([P, RG, SW], bf16))
        L2 = en(nc.sbuf_tensor([P, 2, RG, SW], bf16))
        Epad = en(nc.sbuf_tensor([P, BL, EW], bf16))
        T1 = en(nc.sbuf_tensor([P, 32, EW], bf16))
        T2 = en(nc.sbuf_tensor([P, 16, EW], bf16))
        f2b = en(nc.sbuf_tensor([P, 1, EW], bf16))
        ecb = en(nc.sbuf_tensor([P, BL], bf16))
        fo = en(nc.sbuf_tensor([NK, BL], f32))
        ps = [en(nc.psum_tensor(f"ps{d}", [P, NB], f32)) for d in range(DK)]

        with (
            nc.semaphore("dS") as dS,
            nc.semaphore("sP") as sP,
            nc.semaphore("sA") as sA,
            nc.semaphore("sB") as sB,
            nc.semaphore("s1") as s1,
            nc.semaphore("s2") as s2,
            nc.semaphore("dF") as dF,
            nc.Block() as block,
        ):

            @block.sync
            def _(sync):
                sync.dma_start(
                    kT_all[:], kT_d.rearrange("p (i c) -> p i c", i=IT)
                ).then_inc(dS, 16)
                sync.dma_start(
                    xT_all[:], xT_d.rearrange("p (i c) -> p i c", i=IT)
                ).then_inc(dS, 16)
                sync.wait_ge(dF, 1)
                sync.dma_start(fo_d[:, :], fo[:]).then_inc(dS, 16)
                sync.dma_start(
                    f2_d[:, :], f2b[:].rearrange("p a c -> p (a c)")
                ).then_inc(dS, 16)
                sync.dma_start(ec_d[:, :], ecb[:]).then_inc(dS, 16)
                sync.wait_ge(dS, 80)

            @block.tensor
            def _(tensor):
                tensor.wait_ge(dS, 32)  # both loads
                for d in range(DK):
                    for i in range(IT):
                        mm = nc.tensor.matmul(
                            ps[d][:],
                            kT_all[:, i, NK * d : NK * (d + 1)],
                            xT_all[:, i, :],
                            start=(i == 0),
                            stop=(i == IT - 1),
                        )
                    mm.then_inc(sP, 1)

            @block.scalar
            def _(scalar):
                # zero-init (margins read by the tree) — hidden in preamble
                nc.scalar.memzero(Epad[:, :, 0:64])
                nc.scalar.memzero(Epad[:, :, 320:EW])
                nc.scalar.memzero(T1[:, :, 0:32])
                nc.scalar.memzero(T2[:, :, 0:16])
                nc.scalar.memzero(f2b[:])
                for d in range(DK):
                    scalar.wait_ge(sP, d + 1)
                    nc.scalar.activation(avT[:, d, :], ps[d][:], AF.Copy)
                    act = nc.scalar.activation(
                        avS[:, d, :], ps[d][:, 1:321], AF.Copy
                    )
                act = nc.scalar.activation(
                    avFn4[:], ps[DK - 1][:, 0:BL], AF.Copy, scale=-1.0
                )
                act.then_inc(sA, 1)
                for g in range(NG):
                    if g >= 2:
                        scalar.wait_ge(s1, g - 1)  # t4[g%2] free
                    for r in range(RG):
                        rr = g * RG + r
                        act = nc.scalar.activation(
                            t4[:, g % 2, r, :],
                            avT[:, DK - 1, rr + 1 : rr + 1 + SW],
                            AF.Abs,
                            bias=avFn4[:, rr : rr + 1],
                        )
                    act.then_inc(sB, 1)
                    if g >= 1:
                        scalar.wait_ge(s1, g)  # L2[(g-1)%2] ready
                        for r in range(RG):
                            rr = (g - 1) * RG + r
                            act = nc.scalar.activation(
                                Epad[:, rr, 64 : 64 + SW],
                                L2[:, (g - 1) % 2, r, :],
                                AF.Exp,
                                scale=-1.0,
                                accum_out=fo[:, rr : rr + 1],
                            )
                        act.then_inc(s2, 1)
                scalar.wait_ge(s1, NG)
                for r in range(RG):
                    rr = (NG - 1) * RG + r
                    act = nc.scalar.activation(
                        Epad[:, rr, 64 : 64 + SW],
                        L2[:, (NG - 1) % 2, r, :],
                        AF.Exp,
                        scale=-1.0,
                        accum_out=fo[:, rr : rr + 1],
                    )
                act.then_inc(s2, 1)

            @block.vector
            def _(vector):
                avT_b = avT[:]
                avS_b = avS[:]
                td_b = td[:]
                dup_b = dup[:]

                def tree_l1(k):
                    r0 = 8 * k
                    nc.vector.tensor_tensor(
                        out=T1[:, r0 : r0 + 8, 32:EW],
                        in0=Epad[:, r0 : r0 + 8, 32:EW],
                        in1=Epad[:, r0 + 32 : r0 + 40, 0 : EW - 32],
                        op=OP.add,
                    )

                vector.wait_ge(sA, 1)  # avT/avS ready
                for g in range(NG):
                    if g >= 2:
                        vector.wait_ge(s2, g - 1)  # L2[g%2] free
                    g0 = g * RG
                    nc.vector.tensor_copy(
                        dup_b,
                        avT[:, 0:ND, g0 : g0 + RG]
                        .rearrange("p d r -> p r d")[:, :, :, None]
                        .broadcast_to([P, RG, ND, 2]),
                    )
                    for r in range(RG):
                        rr = g0 + r
                        if rr % 2 == 0:
                            src = avS[:, 0:ND, rr : rr + SW]
                        else:
                            src = avT[:, 0:ND, rr + 1 : rr + 1 + SW]
                        nc.vector.tensor_tensor(
                            out=td[:, r, :, :].rearrange(
                                "p d (b j) -> p d b j", j=2
                            ),
                            in0=src.rearrange("p d (b j) -> p d b j", j=2),
                            in1=dup[:, r, :, None, :].broadcast_to(
                                [P, ND, SW // 2, 2]
                            ),
                            op=OP.subtract,
                        )
                    nc.vector.tensor_scalar(
                        td_b.bitcast(u16),
                        td_b.bitcast(u16),
                        0x7FFF,
                        None,
                        OP.bitwise_and,
                    )
                    nc.vector.tensor_tensor(
                        out=ta[:], in0=td[:, :, 0, :], in1=td[:, :, 1, :],
                        op=OP.add,
                    )
                    nc.vector.tensor_tensor(
                        out=tb[:], in0=td[:, :, 2, :], in1=td[:, :, 3, :],
                        op=OP.add,
                    )
                    nc.vector.tensor_tensor(
                        out=tc[:], in0=ta[:], in1=tb[:], op=OP.add
                    )
                    vector.wait_ge(sB, g + 1)  # t4[g%2] ready
                    nc.vector.tensor_tensor(
                        out=L2[:, g % 2, :, :], in0=tc[:],
                        in1=t4[:, g % 2, :, :], op=OP.add,
                    ).then_inc(s1, 1)
                    if g >= 5:
                        vector.wait_ge(s2, g)  # exps through group g-1
                        tree_l1(g - 5)
                vector.wait_ge(s2, NG)  # Epad complete
                tree_l1(3)
                nc.vector.tensor_tensor(
                    out=T2[:, 0:16, 16:EW], in0=T1[:, 0:16, 16:EW],
                    in1=T1[:, 16:32, 0 : EW - 16], op=OP.add,
                )
                nc.vector.tensor_tensor(
                    out=T1[:, 0:8, 8:EW], in0=T2[:, 0:8, 8:EW],
                    in1=T2[:, 8:16, 0 : EW - 8], op=OP.add,
                )
                nc.vector.tensor_tensor(
                    out=T2[:, 0:4, 4:EW], in0=T1[:, 0:4, 4:EW],
                    in1=T1[:, 4:8, 0 : EW - 4], op=OP.add,
                )
                nc.vector.tensor_tensor(
                    out=T1[:, 0:2, 2:EW], in0=T2[:, 0:2, 2:EW],
                    in1=T2[:, 2:4, 0 : EW - 2], op=OP.add,
                )
                nc.vector.tensor_tensor(
                    out=f2b[:, :, 1:EW], in0=T1[:, 0:1, 1:EW],
                    in1=T1[:, 1:2, 0 : EW - 1], op=OP.add,
                )
                nc.vector.tensor_copy(
                    ecb[:].rearrange("p (c a) -> p c a", a=1),
                    Epad[:, :, 319:320],
                ).then_inc(dF, 1)

    nc.compile()
    return nc


def make_in_maps(x, theta, log_weight_scale, bias):
    import ml_dtypes

    bf = ml_dtypes.bfloat16
    xT = np.ascontiguousarray(x.T).astype(bf)  # [NIN, B]
    l2 = np.sqrt(np.sum(theta.astype(np.float64) ** 2, axis=0))  # [K, D]
    kern = theta * (np.exp(log_weight_scale) / l2)[None]  # [NIN, K, D] f32
    kT = (
        np.ascontiguousarray(kern.transpose(0, 2, 1))
        .reshape(NIN, DK * NK)
        .astype(bf)
    )
    kTlin = np.ascontiguousarray(
        kT.reshape(IT, P, DK * NK).transpose(1, 0, 2).reshape(P, IT * DK * NK)
    )
    maps = []
    for c in range(NCORES):
        xr = np.roll(xT, -BL * c, axis=1)[:, 0:NB]  # [NIN, NB]
        xlin = np.ascontiguousarray(
            xr.reshape(IT, P, NB).transpose(1, 0, 2).reshape(P, IT * NB)
        )
        maps.append({"xTlin": xlin, "kTlin": kTlin})
    return maps


_CACHE = {}


def get_nc():
    if "nc" not in _CACHE:
        _CACHE["nc"] = build_nc()
    return _CACHE["nc"]


def kernel(x, theta, log_weight_scale, bias):
    from concourse.bass_utils import run_bass_kernel_spmd

    x = np.asarray(x, dtype=np.float32)
    theta = np.asarray(theta, dtype=np.float32)
    log_weight_scale = np.asarray(log_weight_scale, dtype=np.float32)
    bias = np.asarray(bias, dtype=np.float32)

    nc = get_nc()
    in_maps = make_in_maps(x, theta, log_weight_scale, bias)
    res = run_bass_kernel_spmd(nc, in_maps, list(range(NCORES))).results

    F = np.zeros((NK, B), dtype=np.float64)
    cols = np.arange(EW)
    for c in range(NCORES):
        F[:, c * BL : (c + 1) * BL] += res[c]["fo"].astype(np.float64)
        g = (c * BL + (cols - 63)) % B
        np.add.at(F, (slice(None), g), res[c]["f2"].astype(np.float64))
        F[:, c * BL : (c + 1) * BL] -= res[c]["ec"].astype(np.float64)
    f = F.T + bias[None, :]  # [B, NK]
    return np.concatenate([x, f.astype(np.float32)], axis=1)


# revision 19
# speedup vs baseline: 4.5727x; 1.1616x over previous
"""Raw-bass v7: symmetric halving + custom fused DVE op.

Same math as v6.x (strips j=1..256 per row; every unordered pair covered
once; distance-256 double count fixed on host via Ecol).

Registers a custom DVE op ABSD2_ANT:
    out = |Src0 - C0| + |Src1 - C1|
(2 tensor streams + 2 per-partition scalars, runs at 1 elem/cycle) which
computes two |actv_d(strip) - actv_d(r)| planes AND their sum in one
instruction — replacing per-plane subtract + sign-clear + one add level.

Per group of RG=8 rows:
  DVE: per row 2x ABSD2 (planes 0+1 -> ta, planes 2+3 -> tb), then
       tc = ta + tb and L = tc + t4 (TT adds at 2x)
  ACT: plane-4 |.| via Abs activation with negated [P,1] bias -> t4;
       per-row Exp(-L) with accum_out -> E strip into Epad + f_own
f2 via the diagonal-shift add tree (level 1 pipelined into groups 5..7).
Host: scatter-add combine as in v6.
"""

import numpy as np
from contextlib import ExitStack

B, NIN, NK, DK = 512, 1024, 128, 5
NCORES = 8
BL = B // NCORES
P = 128
IT = NIN // P
RG = 8
NG = BL // RG
NB = 322          # activation columns needed (strip max col 319, +pad)
SW = 256          # strip width
EW = 384          # Epad row width (data at 64..320)

_ABSD2 = {}


def _get_absd2():
    if "op" in _ABSD2:
        return _ABSD2["op"]
    from concourse.dve_spec import Spec, Src0, Src1, C0, C1, maxx, lower
    from concourse.dve_spec import _has_src1 as has_src1
    from concourse import dve_ops
    from concourse.dve_uop import DveOpSpec

    name = "ABSD2_ANT"
    existing = [op for op in dve_ops.OPS if op.name == name]
    if existing:
        _ABSD2["op"] = existing[0]
        return existing[0]
    spec = Spec(
        body=maxx(Src0 - C0, C0 - Src0) + maxx(Src1 - C1, C1 - Src1),
        reference=lambda in0, in1, s0, s1, imm2: (
            np.abs(in0.astype(np.float32) - s0)
            + np.abs(in1.astype(np.float32) - s1)
        ).astype(np.float32),
    )
    opcode = dve_ops._CUSTOM_DVE_ROW_BASE + len(dve_ops.OPS)
    shas = {}
    for ver in ("v3", "v4"):
        s = DveOpSpec(
            name=name, opcode=opcode, uops=lower(spec, ver=ver),
            rd1_en=has_src1(spec),
        )
        shas[ver] = s.sha(ver)
    op = dve_ops.DveOp(name, spec, subdim=False, uops_sha=shas)
    dve_ops.OPS.append(op)
    dve_ops._SUB_OPCODE_FOR_NAME[name] = opcode
    dve_ops.CUSTOM_DVE_SPECS[name] = spec
    _ABSD2["op"] = op
    return op


def build_nc():
    import concourse.bacc as bacc
    import concourse.mybir as mybir

    f32 = mybir.dt.float32
    bf16 = mybir.dt.bfloat16
    AF = mybir.ActivationFunctionType
    OP = mybir.AluOpType
    absd2 = _get_absd2()

    nc = bacc.Bacc(None, target_bir_lowering=False)
    xT_d = nc.declare_dram_parameter("xTlin", [P, IT * NB], bf16, isOutput=False)
    kT_d = nc.declare_dram_parameter("kTlin", [P, IT * DK * NK], bf16, isOutput=False)
    fo_d = nc.declare_dram_parameter("fo", [NK, BL], f32, isOutput=True)
    f2_d = nc.declare_dram_parameter("f2", [NK, EW], bf16, isOutput=True)
    ec_d = nc.declare_dram_parameter("ec", [NK, BL], bf16, isOutput=True)

    with ExitStack() as ctx:
        en = ctx.enter_context
        kT_all = en(nc.sbuf_tensor([P, IT, DK * NK], bf16))
        xT_all = en(nc.sbuf_tensor([P, IT, NB], bf16))
        avT = en(nc.sbuf_tensor([P, DK, NB], bf16))
        avF = en(nc.sbuf_tensor([P, 4, BL], f32))
        avFn4 = en(nc.sbuf_tensor([P, BL], f32))
        t4 = en(nc.sbuf_tensor([P, 2, RG, SW], bf16))
        ta = en(nc.sbuf_tensor([P, RG, SW], bf16))
        tb = en(nc.sbuf_tensor([P, RG, SW], bf16))
        tc = en(nc.sbuf_tensor([P, RG, SW], bf16))
        L2 = en(nc.sbuf_tensor([P, 2, RG, SW], bf16))
        Epad = en(nc.sbuf_tensor([P, BL, EW], bf16))
        T1 = en(nc.sbuf_tensor([P, 32, EW], bf16))
        T2 = en(nc.sbuf_tensor([P, 16, EW], bf16))
        f2b = en(nc.sbuf_tensor([P, 1, EW], bf16))
        ecb = en(nc.sbuf_tensor([P, BL], bf16))
        fo = en(nc.sbuf_tensor([NK, BL], f32))
        ps = [en(nc.psum_tensor(f"ps{d}", [P, NB], f32)) for d in range(DK)]

        with (
            nc.semaphore("dS") as dS,
            nc.semaphore("sP") as sP,
            nc.semaphore("sA") as sA,
            nc.semaphore("sB") as sB,
            nc.semaphore("s1") as s1,
            nc.semaphore("s2") as s2,
            nc.semaphore("dF") as dF,
            nc.Block() as block,
        ):

            @block.sync
            def _(sync):
                sync.dma_start(
                    kT_all[:], kT_d.rearrange("p (i c) -> p i c", i=IT)
                ).then_inc(dS, 16)
                sync.dma_start(
                    xT_all[:], xT_d.rearrange("p (i c) -> p i c", i=IT)
                ).then_inc(dS, 16)
                sync.wait_ge(dF, 1)
                sync.dma_start(fo_d[:, :], fo[:]).then_inc(dS, 16)
                sync.dma_start(
                    f2_d[:, :], f2b[:].rearrange("p a c -> p (a c)")
                ).then_inc(dS, 16)
                sync.dma_start(ec_d[:, :], ecb[:]).then_inc(dS, 16)
                sync.wait_ge(dS, 80)

            @block.tensor
            def _(tensor):
                tensor.wait_ge(dS, 32)  # both loads
                for d in range(DK):
                    for i in range(IT):
                        mm = nc.tensor.matmul(
                            ps[d][:],
                            kT_all[:, i, NK * d : NK * (d + 1)],
                            xT_all[:, i, :],
                            start=(i == 0),
                            stop=(i == IT - 1),
                        )
                    mm.then_inc(sP, 1)

            @block.scalar
            def _(scalar):
                # zero-init (margins read by the tree) — hidden in preamble
                nc.scalar.memzero(Epad[:, :, 0:64])
                nc.scalar.memzero(Epad[:, :, 320:EW])
                nc.scalar.memzero(T1[:, :, 0:32])
                nc.scalar.memzero(T2[:, :, 0:16])
                nc.scalar.memzero(f2b[:])
                for d in range(4):  # planes 0..3 -> DVE customs
                    scalar.wait_ge(sP, d + 1)
                    nc.scalar.activation(avT[:, d, :], ps[d][:], AF.Copy)
                    act = nc.scalar.activation(
                        avF[:, d, :], ps[d][:, 0:BL], AF.Copy
                    )
                act.then_inc(sA, 1)
                scalar.wait_ge(sP, 5)  # plane 4 -> ACT abs
                nc.scalar.activation(avT[:, 4, :], ps[4][:], AF.Copy)
                nc.scalar.activation(
                    avFn4[:], ps[4][:, 0:BL], AF.Copy, scale=-1.0
                )
                for g in range(NG):
                    if g >= 2:
                        scalar.wait_ge(s1, g - 1)  # t4[g%2] free
                    for r in range(RG):
                        rr = g * RG + r
                        act = nc.scalar.activation(
                            t4[:, g % 2, r, :],
                            avT[:, 4, rr + 1 : rr + 1 + SW],
                            AF.Abs,
                            bias=avFn4[:, rr : rr + 1],
                        )
                    act.then_inc(sB, 1)
                    if g >= 1:
                        scalar.wait_ge(s1, g)  # L2[(g-1)%2] ready
                        for r in range(RG):
                            rr = (g - 1) * RG + r
                            act = nc.scalar.activation(
                                Epad[:, rr, 64 : 64 + SW],
                                L2[:, (g - 1) % 2, r, :],
                                AF.Exp,
                                scale=-1.0,
                                accum_out=fo[:, rr : rr + 1],
                            )
                        act.then_inc(s2, 1)
                scalar.wait_ge(s1, NG)
                for r in range(RG):
                    rr = (NG - 1) * RG + r
                    act = nc.scalar.activation(
                        Epad[:, rr, 64 : 64 + SW],
                        L2[:, (NG - 1) % 2, r, :],
                        AF.Exp,
                        scale=-1.0,
                        accum_out=fo[:, rr : rr + 1],
                    )
                act.then_inc(s2, 1)

            @block.vector
            def _(vector):
                def tree_l1(k):
                    r0 = 8 * k
                    nc.vector.tensor_tensor(
                        out=T1[:, r0 : r0 + 8, 32:EW],
                        in0=Epad[:, r0 : r0 + 8, 32:EW],
                        in1=Epad[:, r0 + 32 : r0 + 40, 0 : EW - 32],
                        op=OP.add,
                    )

                vector.wait_ge(sA, 1)  # avT/avF planes 0..3 ready
                for g in range(NG):
                    if g >= 2:
                        vector.wait_ge(s2, g - 1)  # L2[g%2] free
                    g0 = g * RG
                    for r in range(RG):
                        rr = g0 + r
                        nc.vector._custom_dve(
                            absd2,
                            out=ta[:, r : r + 1, :].rearrange(
                                "p a c -> p (a c)"
                            ),
                            in0=avT[:, 0, rr + 1 : rr + 1 + SW],
                            in1=avT[:, 1, rr + 1 : rr + 1 + SW],
                            s0=avF[:, 0, rr : rr + 1],
                            s1=avF[:, 1, rr : rr + 1],
                        )
                        nc.vector._custom_dve(
                            absd2,
                            out=tb[:, r : r + 1, :].rearrange(
                                "p a c -> p (a c)"
                            ),
                            in0=avT[:, 2, rr + 1 : rr + 1 + SW],
                            in1=avT[:, 3, rr + 1 : rr + 1 + SW],
                            s0=avF[:, 2, rr : rr + 1],
                            s1=avF[:, 3, rr : rr + 1],
                        )
                    nc.vector.tensor_tensor(
                        out=tc[:], in0=ta[:], in1=tb[:], op=OP.add
                    )
                    vector.wait_ge(sB, g + 1)  # t4[g%2] ready
                    nc.vector.tensor_tensor(
                        out=L2[:, g % 2, :, :], in0=tc[:],
                        in1=t4[:, g % 2, :, :], op=OP.add,
                    ).then_inc(s1, 1)
                    if g >= 5:
                        vector.wait_ge(s2, g)  # exps through group g-1
                        tree_l1(g - 5)
                vector.wait_ge(s2, NG)  # Epad complete
                tree_l1(3)
                nc.vector.tensor_tensor(
                    out=T2[:, 0:16, 16:EW], in0=T1[:, 0:16, 16:EW],
                    in1=T1[:, 16:32, 0 : EW - 16], op=OP.add,
                )
                nc.vector.tensor_tensor(
                    out=T1[:, 0:8, 8:EW], in0=T2[:, 0:8, 8:EW],
                    in1=T2[:, 8:16, 0 : EW - 8], op=OP.add,
                )
                nc.vector.tensor_tensor(
                    out=T2[:, 0:4, 4:EW], in0=T1[:, 0:4, 4:EW],
                    in1=T1[:, 4:8, 0 : EW - 4], op=OP.add,
                )
                nc.vector.tensor_tensor(
                    out=T1[:, 0:2, 2:EW], in0=T2[:, 0:2, 2:EW],
                    in1=T2[:, 2:4, 0 : EW - 2], op=OP.add,
                )
                nc.vector.tensor_tensor(
                    out=f2b[:, :, 1:EW], in0=T1[:, 0:1, 1:EW],
                    in1=T1[:, 1:2, 0 : EW - 1], op=OP.add,
                )
                nc.vector.tensor_copy(
                    ecb[:].rearrange("p (c a) -> p c a", a=1),
                    Epad[:, :, 319:320],
                ).then_inc(dF, 1)

    nc.compile()
    return nc


def make_in_maps(x, theta, log_weight_scale, bias):
    import ml_dtypes

    bf = ml_dtypes.bfloat16
    xT = np.ascontiguousarray(x.T).astype(bf)  # [NIN, B]
    l2 = np.sqrt(np.sum(theta.astype(np.float64) ** 2, axis=0))  # [K, D]
    kern = theta * (np.exp(log_weight_scale) / l2)[None]  # [NIN, K, D] f32
    kT = (
        np.ascontiguousarray(kern.transpose(0, 2, 1))
        .reshape(NIN, DK * NK)
        .astype(bf)
    )
    kTlin = np.ascontiguousarray(
        kT.reshape(IT, P, DK * NK).transpose(1, 0, 2).reshape(P, IT * DK * NK)
    )
    maps = []
    for c in range(NCORES):
        xr = np.roll(xT, -BL * c, axis=1)[:, 0:NB]  # [NIN, NB]
        xlin = np.ascontiguousarray(
            xr.reshape(IT, P, NB).transpose(1, 0, 2).reshape(P, IT * NB)
        )
        maps.append({"xTlin": xlin, "kTlin": kTlin})
    return maps


_CACHE = {}


def get_nc():
    if "nc" not in _CACHE:
        _CACHE["nc"] = build_nc()
    return _CACHE["nc"]


def kernel(x, theta, log_weight_scale, bias):
    from concourse.bass_utils import run_bass_kernel_spmd

    x = np.asarray(x, dtype=np.float32)
    theta = np.asarray(theta, dtype=np.float32)
    log_weight_scale = np.asarray(log_weight_scale, dtype=np.float32)
    bias = np.asarray(bias, dtype=np.float32)

    nc = get_nc()
    in_maps = make_in_maps(x, theta, log_weight_scale, bias)
    res = run_bass_kernel_spmd(nc, in_maps, list(range(NCORES))).results

    F = np.zeros((NK, B), dtype=np.float64)
    cols = np.arange(EW)
    for c in range(NCORES):
        F[:, c * BL : (c + 1) * BL] += res[c]["fo"].astype(np.float64)
        g = (c * BL + (cols - 63)) % B
        np.add.at(F, (slice(None), g), res[c]["f2"].astype(np.float64))
        F[:, c * BL : (c + 1) * BL] -= res[c]["ec"].astype(np.float64)
    f = F.T + bias[None, :]  # [B, NK]
    return np.concatenate([x, f.astype(np.float32)], axis=1)
